# revision 13
# baseline (speedup 1.0000x reference)
"""Trainium2 Bass kernel for nn_CA1Replace: 1D cellular automaton
(rule 110, low-bit-first lookup), 32 rows x 16384 cells, 64 iterations,
all 65 states returned as [32, 65, 16384] int32.

Sharding: pure data parallelism - 4 rows per NeuronCore across 8 cores.

Per-core algorithm (2 DoubleRow matmuls + 1 DVE op per iteration):

  State layout [128, 520] fp8: row-pair g = r//2 occupies columns
  [260g, 260g+260); within a pair, cell w of row r sits at column
  260g + 2 + 2*(w//128) + r%2, partition w%128 (rows interleaved so the
  matmul moving AP is 3D); cols 260g+{0,1} and 260g+{258,259} are zero
  pads implementing the wrap=False row edges.

  The rule s' = lookup[L + 2C + 4R] (rule 110) is computed as
      v  = -4L + C - 3R            (PE matmul, fp8 DoubleRow)
      s' = (v >= -3.5) == C        (single DVE scalar_tensor_tensor)
  which is exact for all 8 neighborhoods.

  The matmul contracts over 256-cell windows (2 k-tiles = 2 adjacent
  state columns) in fp8 DoubleRow mode (0.5 cycles/row). Two stationaries:
  W_hi (M=128, columns 0:64 zero - DoubleRow rejects sub-tile positions,
  so the hi matmul writes the full partition range with zeros below)
  produces cells 128(k-1)+64+m -> psum partitions 64:128 with start=True;
  W_lo (M=64) accumulates cells 128k+m into partitions 0:64 (start=False).
  The zero-pad columns make the row edges (wrap=False) fall out naturally.

  Work is split into 2 chunks (row pairs) so the DVE drain of chunk c
  overlaps the PE matmuls of the other chunk / next iteration. All 65
  states accumulate in an SBUF history slab, DMA'd out in chunks.
"""

import numpy as np
import ml_dtypes

import concourse.bass as bass
import concourse.mybir as mybir
from concourse.ap import AP
from concourse.tile import TileContext
from concourse.vector_clock import ScopedClock
from concourse.bass_utils import run_bass_kernel_spmd

B, W, ITERS, NCORES = 32, 16384, 64, 8
NT = ITERS + 1
RPC = B // NCORES          # 4 rows per core
GCOLS = 260                # columns per row-pair (2*128 cells + 2*2 pads)
SCOLS = (RPC // 2) * GCOLS  # 520 state columns per state
NSEG = W // 128            # 128 cell-columns per row

_f32 = mybir.dt.float32
_bf16 = mybir.dt.bfloat16
_fp8 = mybir.dt.float8e4
_f8np = ml_dtypes.float8_e4m3
AO = mybir.AluOpType
AF = mybir.ActivationFunctionType
PM = mybir.MatmulPerfMode

DMA_CHUNK = 4   # states per output DMA
NCHUNK = RPC // 2          # row-pair chunks per iteration (2)
PCOLS = 256                # psum cols per chunk

WL, WC, WR = -4.0, 1.0, -3.0   # v = -4L + C - 3R ; s' = (v >= -3.5) == C
THR = -3.5


def _patch_tile_drain():
    """This walrus build accepts at most ONE sync-wait per CTRL
    instruction; Tile's kernel-tail drain accumulates one wait per used
    processor. Split the extra waits onto dedicated nops."""
    if getattr(TileContext, "_drain_patched", False):
        return

    def _drain_and_barrier(self, tick_clock, wait_clock):
        nc = self.nc
        drain_inst = nc.sync.drain()
        wait_clock.add_sem_waits(
            drain_inst.ins, ScopedClock({None: tick_clock.global_clock})
        )
        si = drain_inst.ins.sync_info
        waits = list(si.on_wait or [])
        upd = list(si.on_update or [])
        if len(waits) > 1:
            drain_inst.ins.sync_info = mybir.SyncInfo(on_wait=waits[:1], on_update=upd)
            for w in waits[1:]:
                nop_inst = nc.sync.nop()
                nop_inst.ins.sync_info = mybir.SyncInfo(on_wait=[w], on_update=[])
        nc.all_engine_barrier()
        assert self.sems is not None
        popped = nc._tile_sem_poison_stack.pop()
        assert popped is self._sem_poison
        nc.clear_and_free_semaphores(list(self.sems.allocated().values()))
        nc.all_engine_barrier()

    TileContext._drain_and_barrier = _drain_and_barrier
    TileContext._drain_patched = True


def _legalize_sync_waits(nc):
    """Hoist extra sync-waits (walrus allows one per instruction) onto
    fresh same-engine nops inserted directly before the offender; the
    engine is in-order so serializing the waits is equivalent."""
    for f in nc.m.functions:
        for bb in f.blocks:
            insts = list(bb.instructions)
            new_list = []
            changed = False
            for ins in insts:
                si = ins.sync_info
                if si is not None and si.on_wait and len(si.on_wait) > 1:
                    changed = True
                    waits = list(si.on_wait)
                    eng = ins.engine
                    for w in waits[:-1]:
                        h = nc.engines[eng].nop()
                        nop_ins = h.ins
                        nop_ins.sync_info = mybir.SyncInfo(on_wait=[w], on_update=[])
                        new_list.append(nop_ins)
                    ins.sync_info = mybir.SyncInfo(
                        on_wait=[waits[-1]], on_update=list(si.on_update or [])
                    )
                new_list.append(ins)
            if changed:
                appended = {id(x) for x in new_list} - {id(x) for x in insts}
                for f2 in nc.m.functions:
                    for bb2 in f2.blocks:
                        cur = list(bb2.instructions)
                        stripped = [
                            x for x in cur if not (id(x) in appended and bb2 is not bb)
                        ]
                        if bb2 is bb:
                            bb2.instructions = new_list
                        elif len(stripped) != len(cur):
                            bb2.instructions = stripped


def _build():
    _patch_tile_drain()
    nc = bass.Bass("TRN2", target_bir_lowering=False, debug=False)
    s0 = nc.dram_tensor("s0", [128, SCOLS], _fp8, kind="ExternalInput")
    wts = nc.dram_tensor("wts", [128, 384], _fp8, kind="ExternalInput")
    out = nc.dram_tensor("out", [128, NT * SCOLS], _fp8, kind="ExternalOutput")

    with TileContext(nc) as tc:
        with (
            tc.tile_pool(name="cst", bufs=1) as cst,
            tc.tile_pool(name="hist", bufs=1) as hp,
            tc.tile_pool(name="ps", bufs=2 * NCHUNK, space="PSUM") as ps,
        ):
            wt = cst.tile([128, 384], _fp8, tag="wts")
            nc.sync.dma_start(wt[:, :], wts[:, :])
            W_hi = wt[:, 0:256].rearrange("p (kt m) -> p kt m", kt=2)   # M=128 padded
            W_lo = wt[:, 256:384].rearrange("p (kt m) -> p kt m", kt=2)  # M=64

            hist = hp.tile([128, NT * SCOLS], _fp8)

            # s_0 arrives pre-thresholded from the host (pads already zero);
            # chunk 0 on the ACT queue so iteration 1 can start after half
            # the transfer, chunk 1 in parallel on the SP queue behind wts
            nc.scalar.dma_start(hist[:, 0:GCOLS], s0[:, 0:GCOLS])
            nc.sync.dma_start(hist[:, GCOLS:SCOLS], s0[:, GCOLS:SCOLS])

            # zero the pad columns (4 per row-pair) of states 1..64 once
            padl = AP(hist.tensor, hist.offset + SCOLS,
                      [list(hist.ap[0]), [SCOLS, NT - 1], [GCOLS, NCHUNK], [1, 2]])
            padr = AP(hist.tensor, hist.offset + SCOLS + GCOLS - 2,
                      [list(hist.ap[0]), [SCOLS, NT - 1], [GCOLS, NCHUNK], [1, 2]])
            nc.vector.memset(padl, 0)
            nc.vector.memset(padr, 0)

            def st_ap(t, g):
                """state-t row-pair-g cell columns AP [128, 256]."""
                return AP(hist.tensor, hist.offset + t * SCOLS + g * GCOLS + 2,
                          [list(hist.ap[0]), [1, PCOLS]])

            def rhs_ap(t, g, off):
                """moving AP [128, kt=2, 256] over state-t row-pair g."""
                return AP(hist.tensor, hist.offset + t * SCOLS + g * GCOLS + off,
                          [list(hist.ap[0]), [2, 2], [1, PCOLS]])

            dma_lo = 0
            for t in range(1, NT):
                pss = []
                for c in range(NCHUNK):
                    pst = ps.tile([128, PCOLS], _f32, tag="v")
                    pss.append(pst)
                # per chunk: hi + lo matmuls then drain; chunk c's drain
                # overlaps the other chunk's matmuls / next iteration
                for c in range(NCHUNK):
                    nc.tensor.matmul(
                        pss[c][:, :], W_hi, rhs_ap(t - 1, c, 2),
                        start=True, stop=False, perf_mode=PM.DoubleRow,
                    )
                    nc.tensor.matmul(
                        pss[c][0:64, :], W_lo, rhs_ap(t - 1, c, 0),
                        start=False, stop=True, perf_mode=PM.DoubleRow,
                        skip_group_check=True,
                    )
                    # drain chunk c: s' = (v >= -3.5) == C
                    nc.vector.scalar_tensor_tensor(
                        st_ap(t, c), pss[c][:, :], THR, st_ap(t - 1, c),
                        AO.is_ge, AO.is_equal,
                    )
                if t % DMA_CHUNK == 0 or t == NT - 1:
                    # per-row-pair DMA on separate queues: each half waits
                    # only on its own chunk's drains
                    sh = hist[:, dma_lo * SCOLS: (t + 1) * SCOLS].rearrange(
                        "p (n s) -> p n s", s=SCOLS)
                    so = out[:, dma_lo * SCOLS: (t + 1) * SCOLS].rearrange(
                        "p (n s) -> p n s", s=SCOLS)
                    nc.sync.dma_start(so[:, :, 0:GCOLS], sh[:, :, 0:GCOLS])
                    nc.scalar.dma_start(so[:, :, GCOLS:SCOLS], sh[:, :, GCOLS:SCOLS])
                    dma_lo = t + 1
    _legalize_sync_waits(nc)
    return nc


_nc_cache = None


def _get_nc():
    global _nc_cache
    if _nc_cache is None:
        _nc_cache = _build()
    return _nc_cache


def _weights_np() -> np.ndarray:
    whi = np.zeros((128, 2, 128), np.float32)   # cols 0:64 stay zero (padding)
    wlo = np.zeros((128, 2, 64), np.float32)
    for m in range(64):
        # hi: cell w = 128(k-1) + 64 + m from window k (kt0 = cell-col k-1, kt1 = k)
        whi[63 + m, 0, 64 + m] = WL
        whi[64 + m, 0, 64 + m] = WC
        if m < 63:
            whi[65 + m, 0, 64 + m] = WR
        else:
            whi[0, 1, 64 + m] = WR
        # lo: cell w = 128k + m
        if m == 0:
            wlo[127, 0, m] = WL
        else:
            wlo[m - 1, 1, m] = WL
        wlo[m, 1, m] = WC
        wlo[m + 1, 1, m] = WR
    w = np.concatenate(
        [whi.reshape(128, 256), wlo.reshape(128, 128)], axis=1
    )
    return w.astype(_f8np)


def _prep_core(xc: np.ndarray) -> np.ndarray:
    # s0[p, 260*(r//2) + 2 + 2j + r%2] = (x[r, 128j + p] >= 0.5); pad cols 0
    xp = np.zeros((128, SCOLS), np.float32)
    xr = (xc >= 0.5).reshape(RPC, NSEG, 128).transpose(2, 0, 1)  # [p, r, j]
    for r in range(RPC):
        base = (r // 2) * GCOLS + 2 + (r % 2)
        xp[:, base: base + 2 * NSEG: 2] = xr[:, r, :]
    return xp.astype(_f8np)


def _post_core(o: np.ndarray) -> np.ndarray:
    raw = np.asarray(o)
    if raw.dtype != np.uint8:
        raw = raw.view(np.uint8)
    bits = (raw != 0).astype(np.int32)
    a = bits.reshape(128, NT, RPC // 2, GCOLS)[:, :, :, 2: 2 + 2 * NSEG]
    a = a.reshape(128, NT, RPC // 2, NSEG, 2)      # [p, t, g, j, rp]
    # out[r = 2g+rp, t, 128j+p]
    return a.transpose(2, 4, 1, 3, 0).reshape(RPC, NT, W)


def run_cores(x: np.ndarray, trace: bool = False):
    nc = _get_nc()
    wn = _weights_np()
    in_maps = [
        {
            "s0": _prep_core(np.asarray(x)[RPC * c: RPC * (c + 1)]),
            "wts": wn,
        }
        for c in range(NCORES)
    ]
    return run_bass_kernel_spmd(nc, in_maps, list(range(NCORES)), trace=trace)


def kernel(x: np.ndarray, lookup: np.ndarray) -> np.ndarray:
    # the (v >= -3.5) == C form hardwired in the device kernel implements
    # exactly this lookup table (rule 110, low-bit-first)
    assert np.array_equal(np.asarray(lookup).ravel(), [0, 1, 1, 1, 0, 1, 1, 0])
    res = run_cores(np.asarray(x))
    out = np.stack([_post_core(r["out"]) for r in res.results])
    return out.reshape(B, NT, W).astype(np.int32)
